# revision 1
# baseline (speedup 1.0000x reference)
"""BinaryConv2D forward on 8 Trainium2 NeuronCores.

out = conv2d_same(inputs, sign(clip(kernel)))   (NHWC, HWIO, 3x3, stride 1)

Sharding: data-parallel over batch (32 images -> 4 per core); the 3x3x256x256
kernel is replicated (forward only, no gradient collective needed).

Per-core kernel strategy:
  - sign(w) computed on-device (scalar engine Sign activation) -> bf16 (+-1 exact).
  - fp32 accuracy from bf16 matmuls: x = hi + lo with hi = bf16(x),
    lo = bf16(x - hi); weights are bf16-exact, so 2 bf16 passes reproduce
    fp32-level accuracy with fp32 PSUM accumulation (~2.6e-6 rel err).
  - input NHWC is channel-contiguous in HBM, so a channel-major on-chip
    layout requires a transpose. Images are DMA-loaded into a "padded
    natural" layout: 116 partitions = 2 padded rows of 58 per block, with
    the SAME-padding border pixels on partitions {0,57,58,115} held zero in
    persistent, memset-once tiles. GPSIMD casts hi/lo to bf16; the PE
    transposes each block (1 cyc/row bf16), and each transposed block is one
    contiguous 2-row evict into the channel-major padded image [C, 56x58].
  - conv as 9 shifted matmuls per C_in chunk x {hi,lo}: stationary [cin,cout]
    128x128 sign-weight tiles; moving operand = row-range of the padded image
    shifted by (dy,dx); x-shifts read the zero pad columns, row ranges are
    clipped per tap (no wasted MACs). PSUM block = 8 output rows (N=448),
    36 accumulating matmuls per block; conv uses 2 PSUM banks, input
    transposes 4, output transposes 2.
  - output PSUM [cout, pix] -> SBUF -> PE fp32 transpose (112-pixel blocks)
    -> [pix, cout] -> natural DMA store to NHWC.

Measured on 8 axon-tunneled trn2 cores with the loop-slope method (tc.For_i
around the body, wall-clock slope over N iterations): ~515-575 us HW exec per
core (4 images) across rounds -- the axon transport drifts ~10% between
measurement rounds; the best low-noise round gave 517 us. Cost-model estimate
450.4 us; PE busy ~419 us of which 372 us is the irreducible 2-pass bf16 conv
stream at 2.4 GHz and ~45 us the layout transposes. Includes ~90 warmup
matmuls at t=0 so the PE HAM clock-gate reaches 8/8 before real work.
"""

import numpy as np

P = 128
H = 56
W = 56
C = 256
XW = W + 2                   # padded row width (zero col at x=-1 and x=56)
NCORES = 8
NTOT = 32
NI = NTOT // NCORES          # images per core
NPIX = H * W                 # 3136
RB = 8                       # output rows per psum block
NT = H // RB                 # 7 psum blocks
TB = 112                     # output pixels per store block (= 2 rows)
TBP = 2 * XW                 # padded-row partitions per input transpose block
NBLK = NPIX // TB            # 28 blocks exactly

_cache = {}


def _build_bass(ni=NI, loops=1):
    import concourse.bacc as bacc
    import concourse.mybir as mybir
    import concourse.tile as tile
    from concourse.masks import make_identity
    from contextlib import ExitStack

    f32 = mybir.dt.float32
    bf16 = mybir.dt.bfloat16

    nc = bacc.Bacc()
    x = nc.dram_tensor("x", [ni, NPIX, C], f32, kind="ExternalInput")
    w = nc.dram_tensor("w", [3, 3, C, C], f32, kind="ExternalInput")
    y = nc.dram_tensor("y", [ni, NPIX, C], f32, kind="ExternalOutput")

    with ExitStack() as ctx:
        tc = ctx.enter_context(tile.TileContext(nc))
        const = ctx.enter_context(tc.tile_pool(name="const", bufs=1))
        wpool = ctx.enter_context(tc.tile_pool(name="wpool", bufs=1))
        wstage = ctx.enter_context(tc.tile_pool(name="wstage", bufs=1))
        xpool = ctx.enter_context(tc.tile_pool(name="xpool", bufs=1))
        natp = ctx.enter_context(tc.tile_pool(name="natp", bufs=2))
        padp = ctx.enter_context(tc.tile_pool(name="padp", bufs=2))
        outp = ctx.enter_context(tc.tile_pool(name="outp", bufs=2))
        psc = ctx.enter_context(tc.tile_pool(name="psc", bufs=3, space="PSUM"))
        psb = ctx.enter_context(tc.tile_pool(name="psb", bufs=3, space="PSUM"))
        psf = ctx.enter_context(tc.tile_pool(name="psf", bufs=2, space="PSUM"))

        identf = const.tile([P, P], f32)
        make_identity(nc, identf)
        identb = const.tile([P, P], bf16)
        make_identity(nc, identb)

        # HAM warmup: ~90 dummy matmuls keep the PE busy from t~0.5us while
        # the first image loads, so the activity monitor lifts the clock gate
        # to 8/8 (2.4 GHz) before the real transposes/convs arrive. Results
        # are never read; shares the ptb psum rotation.
        warm = psc.tile([P, RB * W], f32, name="ps")
        for _ in range(90):
            nc.tensor.matmul(
                warm[:, :P], lhsT=identb, rhs=identb, start=True, stop=True
            )

        # natural tiles hold 2 padded rows (2x58=116) per 128-pixel block:
        # partitions {0,57,58,115} stay zero (SAME-pad borders), row pixels
        # land at [1:57] and [59:115]. The PE transpose of a block then yields
        # two complete padded rows, evicted as one contiguous 2D copy with no
        # per-tile memset of the pad layout. The tiles are persistent and
        # manually double-buffered so the border zeros are written only once.
        xnats = []
        for i in range(2):
            t = xpool.tile([P, NBLK, P], f32, name=f"xnat{i}")
            nc.vector.memset(t, 0.0)
            xnats.append(t)

        # ---- binarized weight tiles: sign(w) as [cin, cout] bf16 ----
        # f32 staging via HWDGE keeps the Pool queue free for the first
        # image's casts (the SWDGE cast-load variant stalled the ramp).
        wst = wstage.tile([P, 9, 2, C], f32, name="wst")
        nc.scalar.dma_start(
            out=wst,
            in_=w[:, :, :, :].rearrange("ky kx (cc p) o -> p (ky kx) cc o", p=P),
        )
        wsgn = {}
        for ky in range(3):
            for kx in range(3):
                for cc in range(2):
                    for oc in range(2):
                        wt = wpool.tile([P, P], bf16, name=f"w_{ky}_{kx}_{cc}_{oc}")
                        nc.scalar.sign(
                            out=wt,
                            in_=wst[:, 3 * ky + kx, cc, P * oc : P * (oc + 1)],
                        )
                        wsgn[(ky, kx, cc, oc)] = wt

        def _images():
            for img in range(ni):
                _one_image(img)

        def _one_image(img):
            # ---- input: load natural, split hi/lo, PE-transpose into padded ----
            pad_tiles = {}
            for cc in range(2):
                xnat = xnats[cc]
                hin = natp.tile([P, NBLK, P], bf16, name="hin")
                lon = natp.tile([P, NBLK, P], bf16, name="lon")
                # chunked loads/casts so the first transposes start after
                # ~1/4 of the image load instead of the whole transfer
                xrows = x[img, :, P * cc : P * (cc + 1)].rearrange(
                    "(b two xx) c -> xx b two c", two=2, xx=W
                )
                for q in range(4):
                    b0, b1 = 7 * q, 7 * (q + 1)
                    nc.sync.dma_start(
                        out=xnat[1 : 1 + W, b0:b1], in_=xrows[:, b0:b1, 0]
                    )
                    nc.sync.dma_start(
                        out=xnat[59 : 59 + W, b0:b1], in_=xrows[:, b0:b1, 1]
                    )
                    nc.gpsimd.tensor_copy(
                        out=hin[:TBP, b0:b1], in_=xnat[:TBP, b0:b1]
                    )
                    nc.gpsimd.tensor_sub(
                        out=lon[:TBP, b0:b1],
                        in0=xnat[:TBP, b0:b1],
                        in1=hin[:TBP, b0:b1],
                    )
                for tag, nat in (("hi", hin), ("lo", lon)):
                    xp = padp.tile([P, H, XW], bf16, name=f"{tag}p{cc}")
                    for b in range(NBLK):
                        pt = psb.tile([P, P], bf16, name="ptb")
                        nc.tensor.transpose(
                            pt[:, :TBP], nat[:TBP, b, :], identb[:TBP, :TBP]
                        )
                        nc.vector.tensor_copy(
                            out=xp[:, 2 * b : 2 * b + 2, :],
                            in_=pt[:, :TBP],
                        )
                    pad_tiles[(tag, cc)] = xp

            # ---- conv matmuls: block-outer, 36 accumulating matmuls each ----
            combos = [
                (cc, ky, kx, tag)
                for tag in ("hi", "lo")
                for cc in range(2)
                for ky in (1, 0, 2)
                for kx in range(3)
            ]
            n_c = len(combos)
            for oc in range(2):
                ocmp = outp.tile([P, NPIX], f32, name="ocmp")
                for t in range(NT):
                    ps = psc.tile([P, RB * W], f32, name="ps")
                    for ci, (cc, ky, kx, tag) in enumerate(combos):
                        dy, dx = ky - 1, kx - 1
                        src = pad_tiles[(tag, cc)]
                        y0 = max(RB * t, -dy)
                        y1 = min(RB * t + RB, H - max(dy, 0))
                        nc.tensor.matmul(
                            ps[:, (y0 - RB * t) * W : (y1 - RB * t) * W],
                            lhsT=wsgn[(ky, kx, cc, oc)],
                            rhs=src[:, y0 + dy : y1 + dy, 1 + dx : 1 + dx + W],
                            start=(ci == 0),
                            stop=(ci == n_c - 1),
                        )
                    nc.vector.tensor_copy(
                        out=ocmp[:, RB * W * t : RB * W * (t + 1)], in_=ps
                    )

                # ---- transpose back to pixel-major, store ----
                HB = NBLK // 2
                for bh in range(2):
                    onat = outp.tile([P, HB, P], f32, name="onat")
                    for bi in range(HB):
                        b = bh * HB + bi
                        pt = psf.tile([P, P], f32, name="ptf")
                        nc.tensor.transpose(
                            pt[:TB], ocmp[:, TB * b : TB * (b + 1)], identf
                        )
                        nc.scalar.copy(out=onat[:TB, bi, :], in_=pt[:TB, :])
                    nc.sync.dma_start(
                        out=y[
                            img,
                            TB * HB * bh : TB * HB * (bh + 1),
                            P * oc : P * (oc + 1),
                        ].rearrange("(b p) c -> p b c", p=TB),
                        in_=onat[:TB],
                    )

        if loops == 1:
            _images()
        else:
            with tc.For_i(0, loops, 1):
                _images()
    nc.compile()
    return nc


def get_bass(ni=NI, loops=1):
    key = (ni, loops)
    if key not in _cache:
        _cache[key] = _build_bass(ni, loops)
    return _cache[key]


def run(inputs, kernel, trace=False, **kw):
    from concourse.bass_utils import run_bass_kernel_spmd

    nc = get_bass()
    xs = np.ascontiguousarray(inputs, dtype=np.float32).reshape(NTOT, NPIX, C)
    wf = np.ascontiguousarray(kernel, dtype=np.float32)
    in_maps = [
        {"x": xs[i * NI : (i + 1) * NI], "w": wf} for i in range(NCORES)
    ]
    res = run_bass_kernel_spmd(nc, in_maps, core_ids=list(range(NCORES)),
                               trace=trace, **kw)
    out = np.concatenate([r["y"] for r in res.results], axis=0)
    return out.reshape(NTOT, H, W, C), res


def kernel(**inputs):
    out, _ = run(inputs["inputs"], inputs["kernel"])
    return out



# revision 2
# speedup vs baseline: 1.8865x; 1.8865x over previous
"""BinaryConv2D forward on 8 Trainium2 NeuronCores.

out = conv2d_same(inputs, sign(clip(kernel)))   (NHWC, HWIO, 3x3, stride 1)

Sharding: data-parallel over batch (32 images -> 4 per core); the 3x3x256x256
kernel is replicated (forward only, no gradient collective needed).

Per-core kernel strategy:
  - sign(w) computed on-device (scalar engine Sign activation) -> bf16 (+-1 exact).
  - fp32 accuracy from bf16 matmuls: x = hi + lo with hi = bf16(x),
    lo = bf16(x - hi); weights are bf16-exact, so 2 bf16 passes reproduce
    fp32-level accuracy with fp32 PSUM accumulation (~2.6e-6 rel err).
  - input NHWC is channel-contiguous in HBM, so a channel-major on-chip
    layout requires a transpose. Images are DMA-loaded into a "padded
    natural" layout: 116 partitions = 2 padded rows of 58 per block, with
    the SAME-padding border pixels on partitions {0,57,58,115} held zero in
    persistent, memset-once tiles. GPSIMD casts hi/lo to bf16; the PE
    transposes each block (1 cyc/row bf16), and each transposed block is one
    contiguous 2-row evict into the channel-major padded image [C, 56x58].
  - conv as 9 shifted matmuls per C_in chunk x {hi,lo}: stationary [cin,cout]
    128x128 sign-weight tiles; moving operand = row-range of the padded image
    shifted by (dy,dx); x-shifts read the zero pad columns, row ranges are
    clipped per tap (no wasted MACs). PSUM block = 8 output rows (N=448),
    36 accumulating matmuls per block; conv uses 2 PSUM banks, input
    transposes 4, output transposes 2.
  - output PSUM [cout, pix] -> SBUF -> PE fp32 transpose (112-pixel blocks)
    -> [pix, cout] -> natural DMA store to NHWC.

Measured on 8 axon-tunneled trn2 cores with the loop-slope method (tc.For_i
around the body, wall-clock slope over N iterations): ~515-575 us HW exec per
core (4 images) across rounds -- the axon transport drifts ~10% between
measurement rounds; the best low-noise round gave 517 us. Cost-model estimate
450.4 us; PE busy ~419 us of which 372 us is the irreducible 2-pass bf16 conv
stream at 2.4 GHz and ~45 us the layout transposes. Includes ~90 warmup
matmuls at t=0 so the PE HAM clock-gate reaches 8/8 before real work.
"""

import numpy as np

P = 128
H = 56
W = 56
C = 256
XW = W + 2                   # padded row width (zero col at x=-1 and x=56)
NCORES = 8
NTOT = 32
NI = NTOT // NCORES          # images per core
NPIX = H * W                 # 3136
RB = 8                       # output rows per psum block
NT = H // RB                 # 7 psum blocks
TB = 112                     # output pixels per store block (= 2 rows)
TBP = 2 * XW                 # padded-row partitions per input transpose block
NBLK = NPIX // TB            # 28 blocks exactly

_cache = {}


def _build_bass(ni=NI, loops=1):
    import concourse.bacc as bacc
    import concourse.mybir as mybir
    import concourse.tile as tile
    from concourse.masks import make_identity
    from contextlib import ExitStack

    f32 = mybir.dt.float32
    bf16 = mybir.dt.bfloat16

    nc = bacc.Bacc()
    x = nc.dram_tensor("x", [ni, NPIX, C], f32, kind="ExternalInput")
    w = nc.dram_tensor("w", [3, 3, C, C], f32, kind="ExternalInput")
    y = nc.dram_tensor("y", [ni, NPIX, C], f32, kind="ExternalOutput")

    with ExitStack() as ctx:
        tc = ctx.enter_context(tile.TileContext(nc))
        const = ctx.enter_context(tc.tile_pool(name="const", bufs=1))
        wpool = ctx.enter_context(tc.tile_pool(name="wpool", bufs=1))
        wstage = ctx.enter_context(tc.tile_pool(name="wstage", bufs=1))
        xpool = ctx.enter_context(tc.tile_pool(name="xpool", bufs=1))
        natp = ctx.enter_context(tc.tile_pool(name="natp", bufs=2))
        padp = ctx.enter_context(tc.tile_pool(name="padp", bufs=2))
        outp = ctx.enter_context(tc.tile_pool(name="outp", bufs=2))
        psc = ctx.enter_context(tc.tile_pool(name="psc", bufs=3, space="PSUM"))
        psb = ctx.enter_context(tc.tile_pool(name="psb", bufs=3, space="PSUM"))
        psf = ctx.enter_context(tc.tile_pool(name="psf", bufs=2, space="PSUM"))

        identf = const.tile([P, P], f32)
        make_identity(nc, identf)
        identb = const.tile([P, P], bf16)
        make_identity(nc, identb)

        # HAM warmup: ~90 dummy matmuls keep the PE busy from t~0.5us while
        # the first image loads, so the activity monitor lifts the clock gate
        # to 8/8 (2.4 GHz) before the real transposes/convs arrive. Results
        # are never read; shares the ptb psum rotation.
        warm = psc.tile([P, RB * W], f32, name="ps")
        for _ in range(90):
            nc.tensor.matmul(
                warm[:, :P], lhsT=identb, rhs=identb, start=True, stop=True
            )

        # natural tiles hold 2 padded rows (2x58=116) per 128-pixel block:
        # partitions {0,57,58,115} stay zero (SAME-pad borders), row pixels
        # land at [1:57] and [59:115]. The PE transpose of a block then yields
        # two complete padded rows, evicted as one contiguous 2D copy with no
        # per-tile memset of the pad layout. The tiles are persistent and
        # manually double-buffered so the border zeros are written only once.
        xnats = []
        for i in range(2):
            t = xpool.tile([P, NBLK, P], f32, name=f"xnat{i}")
            nc.vector.memset(t, 0.0)
            xnats.append(t)

        # ---- binarized weight tiles: sign(w) as [cin, cout] bf16 ----
        # f32 staging via HWDGE keeps the Pool queue free for the first
        # image's casts (the SWDGE cast-load variant stalled the ramp).
        wst = wstage.tile([P, 9, 2, C], f32, name="wst")
        nc.scalar.dma_start(
            out=wst,
            in_=w[:, :, :, :].rearrange("ky kx (cc p) o -> p (ky kx) cc o", p=P),
        )
        wsgn = {}
        for ky in range(3):
            for kx in range(3):
                for cc in range(2):
                    for oc in range(2):
                        wt = wpool.tile([P, P], bf16, name=f"w_{ky}_{kx}_{cc}_{oc}")
                        nc.scalar.sign(
                            out=wt,
                            in_=wst[:, 3 * ky + kx, cc, P * oc : P * (oc + 1)],
                        )
                        wsgn[(ky, kx, cc, oc)] = wt

        def _images():
            for img in range(ni):
                _one_image(img)

        def _one_image(img):
            # ---- input: load natural, cast bf16, PE-transpose into padded ----
            pad_tiles = {}
            for cc in range(2):
                xnat = xnats[cc]
                hin = natp.tile([P, NBLK, P], bf16, name="hin")
                # chunked loads/casts so the first transposes start after
                # ~1/4 of the image load instead of the whole transfer
                xrows = x[img, :, P * cc : P * (cc + 1)].rearrange(
                    "(b two xx) c -> xx b two c", two=2, xx=W
                )
                for q in range(4):
                    b0, b1 = 7 * q, 7 * (q + 1)
                    nc.sync.dma_start(
                        out=xnat[1 : 1 + W, b0:b1], in_=xrows[:, b0:b1, 0]
                    )
                    nc.sync.dma_start(
                        out=xnat[59 : 59 + W, b0:b1], in_=xrows[:, b0:b1, 1]
                    )
                    nc.gpsimd.tensor_copy(
                        out=hin[:TBP, b0:b1], in_=xnat[:TBP, b0:b1]
                    )
                xp = padp.tile([P, H, XW], bf16, name=f"hp{cc}")
                for b in range(NBLK):
                    pt = psb.tile([P, P], bf16, name="ptb")
                    nc.tensor.transpose(
                        pt[:, :TBP], hin[:TBP, b, :], identb[:TBP, :TBP]
                    )
                    nc.vector.tensor_copy(
                        out=xp[:, 2 * b : 2 * b + 2, :],
                        in_=pt[:, :TBP],
                    )
                pad_tiles[cc] = xp

            # ---- conv matmuls: block-outer, 18 accumulating matmuls each ----
            combos = [
                (cc, ky, kx)
                for cc in range(2)
                for ky in (1, 0, 2)
                for kx in range(3)
            ]
            n_c = len(combos)
            for oc in range(2):
                ocmp = outp.tile([P, NPIX], bf16, name="ocmp")
                for t in range(NT):
                    ps = psc.tile([P, RB * W], f32, name="ps")
                    for ci, (cc, ky, kx) in enumerate(combos):
                        dy, dx = ky - 1, kx - 1
                        src = pad_tiles[cc]
                        y0 = max(RB * t, -dy)
                        y1 = min(RB * t + RB, H - max(dy, 0))
                        nc.tensor.matmul(
                            ps[:, (y0 - RB * t) * W : (y1 - RB * t) * W],
                            lhsT=wsgn[(ky, kx, cc, oc)],
                            rhs=src[:, y0 + dy : y1 + dy, 1 + dx : 1 + dx + W],
                            start=(ci == 0),
                            stop=(ci == n_c - 1),
                        )
                    nc.vector.tensor_copy(
                        out=ocmp[:, RB * W * t : RB * W * (t + 1)], in_=ps
                    )

                # ---- transpose back to pixel-major (bf16), store f32 ----
                HB = NBLK // 2
                for bh in range(2):
                    onat = outp.tile([P, HB, P], f32, name="onat")
                    for bi in range(HB):
                        b = bh * HB + bi
                        pt = psf.tile([P, P], bf16, name="ptf")
                        nc.tensor.transpose(
                            pt[:TB], ocmp[:, TB * b : TB * (b + 1)], identb
                        )
                        nc.scalar.copy(out=onat[:TB, bi, :], in_=pt[:TB, :])
                    nc.sync.dma_start(
                        out=y[
                            img,
                            TB * HB * bh : TB * HB * (bh + 1),
                            P * oc : P * (oc + 1),
                        ].rearrange("(b p) c -> p b c", p=TB),
                        in_=onat[:TB],
                    )

        if loops == 1:
            _images()
        else:
            with tc.For_i(0, loops, 1):
                _images()
    nc.compile()
    return nc


def get_bass(ni=NI, loops=1):
    key = (ni, loops)
    if key not in _cache:
        _cache[key] = _build_bass(ni, loops)
    return _cache[key]


def run(inputs, kernel, trace=False, **kw):
    from concourse.bass_utils import run_bass_kernel_spmd

    nc = get_bass()
    xs = np.ascontiguousarray(inputs, dtype=np.float32).reshape(NTOT, NPIX, C)
    wf = np.ascontiguousarray(kernel, dtype=np.float32)
    in_maps = [
        {"x": xs[i * NI : (i + 1) * NI], "w": wf} for i in range(NCORES)
    ]
    res = run_bass_kernel_spmd(nc, in_maps, core_ids=list(range(NCORES)),
                               trace=trace, **kw)
    out = np.concatenate([r["y"] for r in res.results], axis=0)
    return out.reshape(NTOT, H, W, C), res


def kernel(**inputs):
    out, _ = run(inputs["inputs"], inputs["kernel"])
    return out



# revision 58
# speedup vs baseline: 3.1763x; 1.6837x over previous
"""BinaryConv2D forward on 8 Trainium2 NeuronCores.

out = conv2d_same(inputs, sign(clip(kernel)))   (NHWC, HWIO, 3x3, stride 1)

Sharding: data-parallel over batch (32 images -> 4 per core); the 3x3x256x256
kernel is replicated (forward only, no gradient collective needed).

Per-core kernel strategy:
  - sign(w) computed on-device from a bf16 cast-load of the kernel (gpsimd
    SWDGE casts f32->bf16 in flight); two batched Activation sign ops emit
    fp8e4 [cin, pair, cout] stationary tiles (+-1 is exact in fp8).
  - fp8 DoubleRow matmuls: the PE packs 2 fp8 weights per cell, so one
    matmul contracts all 256 input channels (pair dim = channel halves) at
    0.5 cycles/row. Precision comes from a two-level split x = hi + lo
    with hi = fp8(x), lo = fp8(x - hi): both streamed as separate
    accumulating passes, reproducing ~bf16 accuracy (measured ~2e-3 vs the
    2e-2 gate) at half the bf16 cycle count.
  - input NHWC is channel-contiguous in HBM, so the channel-major on-chip
    layout requires a transpose. Images are DMA-loaded into a 112-partition
    natural layout (2 rows of 56 per block), spread across both HWDGE
    queues in quarter-image chunks; casts to bf16 run on Pool (cc=0) and
    DVE (cc=1). The PE transposes each block (1 cyc/row) and per block two
    vector ops derive hi=fp8(x) and lo=fp8(x-hi) straight from the
    transpose PSUM into flat-padded [cin, pair, 58x57] images: 57-wide
    rows where one shared zero column serves as both row r's x=56 pad and
    row r+1's x=-1 pad, so shifted tap windows stay contiguous with only
    one discarded column per row.
  - conv as 9 shifted flat-window DoubleRow matmuls x {hi,lo} per psum
    block of 8 output rows (N=456, pad-row slices clipped at the image
    edges), accumulating 18 matmuls.
  - output PSUM [cout, rows, 58] -> SBUF bf16 (pad cols dropped) -> PE bf16
    transpose into a 7-block PSUM bank -> one batched Activation copy (f32
    upcast) -> natural DMA store per 7-block group, emitted as soon as the
    covering psum blocks are evicted to keep the tail one group deep.
  - a few warmup matmuls at t=0 start the PE p-state ramp clock while the
    first image loads; images are software-pipelined (image i+1's
    transposes interleave between image i's conv psum blocks so the
    in-order PE queue never parks on the hi/lo chains).

Cost-model (CoreSim) estimate 141.8 us per core; measured rel err 3.0e-3
(gate 2e-2; the max-err term is the shared bf16 output rounding).
Lineage: 450.4 us (hi+lo bf16 2-pass baseline) -> 238.8 us (single bf16
pass + bf16 output transposes) -> 230.8 us (dual-queue loads, batched
signs/output stores) -> 148.6 us (fp8 DoubleRow hi/lo + pipelining)
-> 143.9 us (spread transpose drain, deferred output groups, pad-row tap
clipping, fine-grained final stores) -> 141.8 us (57-wide shared-pad-col
row layout, image-0 cast split).
"""

import numpy as np

P = 128
H = 56
W = 56
C = 256
XW = W + 2                   # padded row count (58: rows y=-1..56)
RW = W + 1                   # flat row stride: one shared zero col per row
FL = XW * RW                 # flat padded image length (3306)
FT = 3312                    # fp8 tile free size (junk pad to %16)
NCORES = 8
NTOT = 32
NI = NTOT // NCORES          # images per core
NPIX = H * W                 # 3136
RB = 8                       # output rows per psum block
NT = H // RB                 # 7 psum blocks
TB = 112                     # pixels per transpose block (= 2 rows)
NBLK = NPIX // TB            # 28 blocks exactly
OG = 7                       # output blocks per store group
NG = NBLK // OG              # 4 store groups per oc

_cache = {}


def _build_bass(ni=NI, loops=1, warm=12):
    import concourse.bacc as bacc
    import concourse.mybir as mybir
    import concourse.tile as tile
    from concourse.masks import make_identity
    from contextlib import ExitStack

    f32 = mybir.dt.float32
    bf16 = mybir.dt.bfloat16
    fp8 = mybir.dt.float8e4
    DR = mybir.MatmulPerfMode.DoubleRow

    nc = bacc.Bacc()
    x = nc.dram_tensor("x", [ni, NPIX, C], f32, kind="ExternalInput")
    w = nc.dram_tensor("w", [3, 3, C, C], f32, kind="ExternalInput")
    y = nc.dram_tensor("y", [ni, NPIX, C], f32, kind="ExternalOutput")

    with ExitStack() as ctx:
        tc = ctx.enter_context(tile.TileContext(nc))
        const = ctx.enter_context(tc.tile_pool(name="const", bufs=1))
        wpool = ctx.enter_context(tc.tile_pool(name="wpool", bufs=1))
        wstage = ctx.enter_context(tc.tile_pool(name="wstage", bufs=1))
        natp = ctx.enter_context(tc.tile_pool(name="natp", bufs=2))
        binp = ctx.enter_context(tc.tile_pool(name="binp", bufs=2))
        padp = ctx.enter_context(tc.tile_pool(name="padp", bufs=2))
        ocp = ctx.enter_context(tc.tile_pool(name="ocp", bufs=2))
        onp = ctx.enter_context(tc.tile_pool(name="onp", bufs=4))
        psc = ctx.enter_context(tc.tile_pool(name="psc", bufs=3, space="PSUM"))
        psb = ctx.enter_context(tc.tile_pool(name="psb", bufs=3, space="PSUM"))
        psf = ctx.enter_context(tc.tile_pool(name="psf", bufs=2, space="PSUM"))

        identb = const.tile([P, P], bf16)
        make_identity(nc, identb)

        # ---- binarized weight tiles: sign(w) as fp8 [cin, pair, cout] ----
        wst = wstage.tile([P, 9, 2, C], bf16, name="wst")
        nc.gpsimd.dma_start(
            out=wst,
            in_=w[:, :, :, :].rearrange("ky kx (cc p) o -> p (ky kx) cc o", p=P),
        )
        wsgn = wpool.tile([P, 9, 2, 2, P], fp8, name="wsgn")

        def _sign(oc):
            nc.scalar.sign(
                out=wsgn[:, :, :, oc, :],
                in_=wst[:, :, :, P * oc : P * (oc + 1)],
            )

        # HAM warmup: dummy matmuls keep the PE busy from t~0 while the
        # first image loads, so the p-state ramp reaches full clock before
        # the real transposes/convs arrive. Results are never read.
        wrm = psc.tile([P, RB, RW], f32, name="ps")
        for _ in range(warm):
            nc.tensor.matmul(
                wrm[:, :2, :], lhsT=identb, rhs=identb[:, : 2 * RW],
                start=True, stop=True,
            )

        dmaq = [nc.sync, nc.scalar]

        def _load_image(img):
            # ---- issue loads (both HWDGE queues), bf16 casts (Pool), and
            # allocate the hi/lo fp8 flat-padded images ----
            st = {"img": img, "bins": {}}
            nats = {}
            for cc in range(2):
                nats[cc] = natp.tile([P, NBLK, P], f32, name=f"nat{cc}")
                st["bins"][cc] = binp.tile([P, NBLK, P], bf16, name=f"bin{cc}")
            st["xpb"] = padp.tile([P, 2, NPIX], bf16, name="xpb")
            st["xph"] = padp.tile([P, 2, FT], fp8, name="xph")
            st["xpl"] = padp.tile([P, 2, FT], fp8, name="xpl")
            # zero the SAME-padding borders (rows y=-1,56 and cols x=-1,56)
            # and the junk edge cells some shifted windows read
            for xp8 in (st["xph"], st["xpl"]):
                nc.vector.memset(xp8[:, :, 0:1], 0.0)
                nc.vector.memset(xp8[:, :, 1 + FL : FT], 0.0)
                xv = xp8[:, :, 1 : 1 + FL].rearrange(
                    "p j (r c) -> p j r c", c=RW
                )
                nc.vector.memset(xv[:, :, 0, :], 0.0)
                nc.vector.memset(xv[:, :, XW - 1, :], 0.0)
                nc.vector.memset(xv[:, :, 1 : XW - 1, 0], 0.0)
            # chunked loads/casts; the batched sign ops slot between the
            # first image's chunks
            for q in range(4):
                b0, b1 = 7 * q, 7 * (q + 1)
                for cc in range(2):
                    xnat, hin = nats[cc], st["bins"][cc]
                    xrows = x[img, :, P * cc : P * (cc + 1)].rearrange(
                        "(b two xx) c -> xx b two c", two=2, xx=W
                    )
                    for two in range(2):
                        dmaq[(q + two + cc) % 2].dma_start(
                            out=xnat[W * two : W * (two + 1), b0:b1],
                            in_=xrows[:, b0:b1, two],
                        )
                    # image 0's input is on the critical path: push its
                    # cc=1 casts to DVE so the Pool cast->hi/lo chain
                    # keeps pace with the DMA cadence
                    caster = (
                        nc.vector if (img == 0 and cc == 1) else nc.gpsimd
                    )
                    caster.tensor_copy(
                        out=hin[:TB, b0:b1], in_=xnat[:TB, b0:b1]
                    )
                if img == 0:
                    if q < 2:
                        _sign(q)
                    # image 0 has no previous conv to hide its transposes
                    # under: emit them chunk-by-chunk so the Pool hi/lo
                    # chain starts as soon as each chunk's cast lands
                    for b in range(b0, b1):
                        _emit_transpose(st, 0, b)
                        _emit_transpose(st, 1, b)
            # transpose work-list: early blocks of both cc first, so the
            # next image's first conv group unblocks asap
            st["tlist"] = (
                [] if img == 0 else
                [(cc, b) for b in range(NBLK) for cc in range(2)]
            )
            return st

        def _emit_transpose(st, cc, b):
            # PE-transpose one block, evict to SBUF bf16 (DVE; GPSIMD
            # cannot read PSUM), then derive hi = fp8(x), lo = fp8(x-hi)
            # on Pool into the flat-padded fp8 images
            hin = st["bins"][cc]
            pt = psb.tile([P, P], bf16, name="ptb")
            nc.tensor.transpose(
                pt[:, :TB], hin[:TB, b, :], identb[:TB, :TB]
            )
            bb = st["xpb"][:, cc, TB * b : TB * (b + 1)]
            nc.vector.tensor_copy(out=bb, in_=pt[:, :TB])
            bv = bb.rearrange("p (two xx) -> p two xx", two=2)

            def _dst(xp8):
                return xp8[:, cc, 1 : 1 + FL].rearrange(
                    "p (r c) -> p r c", c=RW
                )[:, 2 * b + 1 : 2 * b + 3, 1 : 1 + W]

            nc.gpsimd.tensor_copy(out=_dst(st["xph"]), in_=bv)
            nc.gpsimd.tensor_sub(out=_dst(st["xpl"]), in0=bv,
                                 in1=_dst(st["xph"]))

        def _drain(st, n):
            for _ in range(n):
                if st and st["tlist"]:
                    _emit_transpose(st, *st["tlist"].pop(0))

        def _conv_image(st, nxt):
            # ---- conv: 18 accumulating DoubleRow matmuls per psum block
            # (hi/lo passes x 9 taps, all 256 cin per matmul); the next
            # image's transposes are interleaved between psum blocks so the
            # in-order PE queue never parks on their hi/lo chains ----
            img = st["img"]
            combos = [
                (st["xph"], ky, kx) for ky in (1, 0, 2) for kx in range(3)
            ] + [
                (st["xpl"], ky, kx) for ky in (1, 0, 2) for kx in range(3)
            ]
            n_c = len(combos)
            # emit each output group as soon as the psum blocks covering
            # its rows are evicted; the final group is only 4 blocks so the
            # post-last-matmul drain is short
            emit_after = {1: [(0, 7)], 3: [(7, 7)], 5: [(14, 7)],
                          6: [(21, 7)]}

            def _emit_group(oc, ocmp, b0, nb, split=False):
                onat = onp.tile([P, OG, P], f32, name="onat")
                pt = psf.tile([P, OG, P], bf16, name="ptf")
                for bi in range(nb):
                    b = b0 + bi
                    nc.tensor.transpose(
                        pt[:TB, bi, :], ocmp[:, TB * b : TB * (b + 1)],
                        identb,
                    )
                nc.scalar.copy(out=onat[:TB, :nb], in_=pt[:TB, :nb])
                parts = ((0, nb // 2), (nb // 2, nb)) if split else ((0, nb),)
                for qi, (p0, p1) in enumerate(parts):
                    dmaq[(b0 + qi) % 2].dma_start(
                        out=y[
                            img,
                            TB * (b0 + p0) : TB * (b0 + p1),
                            P * oc : P * (oc + 1),
                        ].rearrange("(b p) c -> p b c", p=TB),
                        in_=onat[:TB, p0:p1],
                    )

            for oc in range(2):
                # for the very last (image, oc) pair, store fine-grained
                # 4-block groups after every psum block so the post-conv
                # drain is one small group (split across both DMA queues)
                fine = nxt is None and oc == 1
                ea = {t: [(4 * t, 4)] for t in range(NT)} if fine else \
                    emit_after
                pending = []
                ocmp = ocp.tile([P, NPIX], bf16, name="ocmp")
                for t in range(NT):
                    ps = psc.tile([P, RB, RW], f32, name="ps")
                    for ci, (src8, ky, kx) in enumerate(combos):
                        dy, dx = ky - 1, kx - 1
                        fs = (RB * t + dy + 1) * RW + dx + 1
                        # skip the zero pad-row slice of the window for the
                        # edge taps (ci==0 is dy=0, so the start flag still
                        # clears the full region)
                        r0 = 1 if (t == 0 and dy < 0) else 0
                        r1 = RB - (1 if (t == NT - 1 and dy > 0) else 0)
                        nc.tensor.matmul(
                            ps[:, r0:r1, :],
                            lhsT=wsgn[:, 3 * ky + kx, :, oc, :],
                            rhs=src8[:, :, fs + r0 * RW : fs + r1 * RW],
                            start=(ci == 0),
                            stop=(ci == n_c - 1),
                            perf_mode=DR,
                        )
                        # spread the next image's transposes between this
                        # group's matmuls so their psb/DVE-evict chain never
                        # bunches up at the group boundary; flush deferred
                        # output groups a few matmuls in, past the previous
                        # group's eviction latency
                        if ci in (3, 7, 11, 15):
                            _drain(nxt, 1)
                        if ci == 4:
                            for b0, nb, spl in pending:
                                _emit_group(oc, ocmp, b0, nb, spl)
                            pending = []
                    nc.vector.tensor_copy(
                        out=ocmp[:, RB * W * t : RB * W * (t + 1)],
                        in_=ps[:, :, 1 : 1 + W],
                    )
                    for b0, nb in ea.get(t, []):
                        if t == NT - 1:
                            _emit_group(oc, ocmp, b0, nb, fine)
                        else:
                            pending.append((b0, nb, fine))
            _drain(nxt, len(nxt["tlist"]) if nxt else 0)

        def _images():
            st = _load_image(0)
            _drain(st, len(st["tlist"]))
            for img in range(ni):
                nxt = _load_image(img + 1) if img + 1 < ni else None
                _conv_image(st, nxt)
                st = nxt

        if loops == 1:
            _images()
        else:
            with tc.For_i(0, loops, 1):
                _images()
    nc.compile()
    return nc


def get_bass(ni=NI, loops=1):
    key = (ni, loops)
    if key not in _cache:
        _cache[key] = _build_bass(ni, loops)
    return _cache[key]


def run(inputs, kernel, trace=False, **kw):
    from concourse.bass_utils import run_bass_kernel_spmd

    nc = get_bass()
    xs = np.ascontiguousarray(inputs, dtype=np.float32).reshape(NTOT, NPIX, C)
    wf = np.ascontiguousarray(kernel, dtype=np.float32)
    in_maps = [
        {"x": xs[i * NI : (i + 1) * NI], "w": wf} for i in range(NCORES)
    ]
    res = run_bass_kernel_spmd(nc, in_maps, core_ids=list(range(NCORES)),
                               trace=trace, **kw)
    out = np.concatenate([r["y"] for r in res.results], axis=0)
    return out.reshape(NTOT, H, W, C), res


def kernel(**inputs):
    out, _ = run(inputs["inputs"], inputs["kernel"])
    return out
